# revision 38
# baseline (speedup 1.0000x reference)
"""Trainium2 Bass kernel for a single transformer decoder layer
(B=2, S=2048, E=2048, 16 heads, FFN 4x, causal attention, exact gelu,
two layernorms), distributed over 8 NeuronCores.

Sharding:
  - QKV + attention: tensor-parallel over heads (2 heads/core), zero comm.
  - One AllToAll exchanges ctx slices ([head-slice, all tokens] ->
    [all heads, 512-token slice]); each core then runs the fc projection
    with the full Wfc plus LN1 + FFN (full W1/W2) + LN2 on its own
    512-token slice. Host concatenates the 8 output slices.

v2 changes vs baseline:
  - All matmul operands bf16 (LDWEIGHTS halves to ~107ns and hides under
    the matmuls; DMA traffic halves). PSUM accumulation stays f32;
    LN stats/residual paths stay f32/f32r.
  - q/k/v stay resident in SBUF (no DRAM roundtrip between QKV phase and
    attention).
  - Causal trimming: diagonal-block score/ctx/denominator matmuls only
    cover the non-masked column range (PSUM has_written overwrite
    semantics fill the fully-masked region from the mask matmul).
  - reciprocal_approx_fast for softmax/LN denominators (5x faster than
    the exact reciprocal, 18-bit accurate).
  - fc is computed in two token halves so only the first AllToAll is on
    the critical path; the later ones hide under the first half.
  - FFN holds all 64 gelu(h) chunks in SBUF; W2 accumulates 64 chunks in
    a single PSUM group per output tile and LN2 stats interleave with the
    tail of W2.
"""
import functools
import math

import numpy as np
import ml_dtypes

import concourse.bacc as bacc
import concourse.bass as bass
import concourse.mybir as mybir
import concourse.tile as tile
from concourse.bass_utils import run_bass_kernel_spmd

N_CORES = 8
P = 128
B, S, E = 2, 2048, 2048
T = B * S                   # 4096 tokens
NH, HD = 16, 128
FF = 4 * E                  # 8192
KE = E // P                 # 16 contraction chunks
CPC = 2 * HD                # 256 head-dim columns per core
TBLK = T // N_CORES         # 512 tokens per core after the all-to-all
HB = TBLK // 2              # 256-token half-block
EPS = 1e-5

F32 = mybir.dt.float32
F32R = mybir.dt.float32r
BF16 = mybir.dt.bfloat16

Identity = mybir.ActivationFunctionType.Identity
Copy = mybir.ActivationFunctionType.Copy
Exp = mybir.ActivationFunctionType.Exp
Gelu = mybir.ActivationFunctionType.Gelu
Sqrt = mybir.ActivationFunctionType.Sqrt
Square = mybir.ActivationFunctionType.Square
ADD = mybir.AluOpType.add
MULT = mybir.AluOpType.mult
SUB = mybir.AluOpType.subtract


def _ln_finish(nc, pool, psums, x_t, ones, grows_t, gi, g_t, be_t, eps_t,
               out_chunk, mu_ps, sq_ps, tag, chunk_done=None):
    """Finish a layernorm whose sum(x) and sum(x^2) already sit in
    mu_ps/sq_ps [1,512] psums. x_t [128, KE, 512] f32r -> out_chunk(k) APs.
    Apply is 2 DVE passes/chunk: out = (x*g)*bcast(rstd) + be - g*(mu*rstd)."""
    mu_sb = pool.tile([1, 512], F32, tag=f"{tag}_musb", bufs=1, name=f"{tag}_musb")
    nc.scalar.activation(mu_sb[:], mu_ps[:], Copy, scale=1.0 / E)
    m2_sb = pool.tile([1, 512], F32, tag=f"{tag}_m2sb", bufs=1, name=f"{tag}_m2sb")
    nc.scalar.activation(m2_sb[:], sq_ps[:], Copy, scale=1.0 / E)
    var = pool.tile([1, 512], F32, tag=f"{tag}_var", bufs=1, name=f"{tag}_var")
    nc.vector.tensor_mul(var[:], mu_sb[:], mu_sb[:])
    nc.vector.tensor_sub(var[:], m2_sb[:], var[:])
    std = pool.tile([1, 512], F32, tag=f"{tag}_std", bufs=1, name=f"{tag}_std")
    nc.scalar.activation(std[:], var[:], Sqrt, bias=eps_t[:])
    rstd = pool.tile([1, 512], F32, tag=f"{tag}_rstd", bufs=1, name=f"{tag}_rstd")
    nc.vector.reciprocal_approx_fast(rstd[:], std[:])
    rstd_b = pool.tile([1, 512], BF16, tag=f"{tag}_rstdb", bufs=1,
                       name=f"{tag}_rstdb")
    nc.scalar.activation(rstd_b[:], rstd[:], Copy)
    msr = pool.tile([1, 512], BF16, tag=f"{tag}_msr", bufs=1, name=f"{tag}_msr")
    nc.vector.tensor_mul(msr[:], mu_sb[:], rstd[:])
    rbc = psums.tile([P, 512], F32, tag=f"{tag}_rbc", bufs=1, name=f"{tag}_rbc")
    nc.tensor.matmul(rbc[:], ones[0:1, :], rstd_b[:], start=True, stop=True)
    for k in range(KE):
        mbcg = psums.tile([P, 512], F32, tag=f"{tag}_mbcg", bufs=2,
                          name=f"{tag}_mbcg")
        nc.tensor.matmul(
            mbcg[:], grows_t[0:1, gi, k * P:(k + 1) * P],
            msr[:], start=True, stop=True)
        t1 = pool.tile([P, 512], F32, tag=f"{tag}_t1", bufs=2, name=f"{tag}_t1")
        nc.vector.scalar_tensor_tensor(
            t1[:], x_t[:, k, :], g_t[:, k:k + 1], rbc[:], MULT, MULT)
        oc = out_chunk(k)
        nc.vector.scalar_tensor_tensor(
            oc, t1[:], be_t[:, k:k + 1], mbcg[:], ADD, SUB)
        if chunk_done is not None:
            chunk_done(k, oc)
    # (grows row gi,k*128:(k+1)*128 holds g[k*128:(k+1)*128] so
    #  mbcg = g_e * (mu*rstd)_t)


def _build_program(ln_trivial=True):
    nc = bacc.Bacc("TRN2", target_bir_lowering=False, debug=False,
                   num_devices=N_CORES)

    # ---- per-core external inputs ----
    embT_d = nc.dram_tensor("embT", [8, P, KE * 512], BF16, kind="ExternalInput")
    embres_d = nc.dram_tensor("embres", [P, 4 * E], BF16, kind="ExternalInput")
    wq_d = nc.dram_tensor("wq", [P, KE * CPC], BF16, kind="ExternalInput")
    wk_d = nc.dram_tensor("wk", [P, KE * CPC], BF16, kind="ExternalInput")
    wv_d = nc.dram_tensor("wv", [P, KE * CPC], BF16, kind="ExternalInput")
    bqk_d = nc.dram_tensor("bqk", [P, 4], F32, kind="ExternalInput")  # bq|bk chunks
    bvbc_d = nc.dram_tensor("bvbc", [P, CPC], F32, kind="ExternalInput")
    wfc_d = nc.dram_tensor("wfc", [KE, P, E], BF16, kind="ExternalInput")
    vecs_d = nc.dram_tensor("vecs", [P, 6 * KE], F32, kind="ExternalInput")
    # vecs: [bfc | g1 | be1 | b2 | g2 | be2] each [P, KE]
    w1_d = nc.dram_tensor("w1", [64, P, KE * P], BF16, kind="ExternalInput")
    b1_d = nc.dram_tensor("b1", [P, 64], F32, kind="ExternalInput")
    w2_d = nc.dram_tensor("w2", [2, 64, P, 8 * P], BF16, kind="ExternalInput")
    mask_d = nc.dram_tensor("maskT", [P, 4 * 512], BF16, kind="ExternalInput")
    ones_d = nc.dram_tensor("onesblk", [P, P], BF16, kind="ExternalInput")
    eye_d = nc.dram_tensor("eyeblk", [P, P], BF16, kind="ExternalInput")
    onesr_d = nc.dram_tensor("onesr", [P, P], F32R, kind="ExternalInput")
    grows_d = nc.dram_tensor("grows", [1, 2 * KE * P], BF16, kind="ExternalInput")

    bfcrow_d = nc.dram_tensor("bfcrow", [1, E], BF16, kind="ExternalInput")
    b2row_d = nc.dram_tensor("b2row", [1, E], BF16, kind="ExternalInput")
    if not ln_trivial:
        g1bc_d = nc.dram_tensor("g1bc", [P, E], BF16, kind="ExternalInput")
        be1bc_d = nc.dram_tensor("be1bc", [P, E], BF16, kind="ExternalInput")
        g2bc_d = nc.dram_tensor("g2bc", [P, E], BF16, kind="ExternalInput")
        be2bc_d = nc.dram_tensor("be2bc", [P, E], BF16, kind="ExternalInput")
    out_d = nc.dram_tensor("outp", [4, P, E], F32, kind="ExternalOutput")

    # ---- internal DRAM (collective payloads only) ----
    a2a0in_d = nc.dram_tensor("a2a0in_i", [N_CORES, CPC, HB], BF16, kind="Internal")
    a2a0out_d = nc.dram_tensor("a2a0out_i", [N_CORES, CPC, HB], BF16, kind="Internal")
    a2a1in_d = nc.dram_tensor("a2a1in_i", [N_CORES, 2, P, HB], BF16, kind="Internal")
    a2a1out_d = nc.dram_tensor("a2a1out_i", [N_CORES, 2, P, HB], BF16, kind="Internal")

    from contextlib import ExitStack

    with tile.TileContext(nc) as tc, ExitStack() as es:
        with (
            tc.tile_pool(name="const", bufs=1) as cpool,
            tc.tile_pool(name="persist", bufs=1) as ppool,
        ):
            ones = cpool.tile([P, P], BF16, name="ones")
            eye = cpool.tile([P, P], BF16, name="eye")
            mask_t = cpool.tile([P, 4, 512], BF16, name="mask_t")
            bqk_t = cpool.tile([P, 4], F32, name="bqk_t")
            bvbc_t = cpool.tile([P, CPC], F32, name="bvbc_t")
            vecs_t = cpool.tile([P, 6, KE], F32, name="vecs_t")
            b1_t = cpool.tile([P, 64], F32, name="b1_t")
            grows_t = cpool.tile([1, 2, KE * P], BF16, name="grows_t")
            onesr = cpool.tile([P, P], F32R, name="onesr")

            def load_consts():
                nc.sync.dma_start(ones[:], ones_d[:])
                nc.sync.dma_start(eye[:], eye_d[:])
                nc.sync.dma_start(
                    mask_t[:], mask_d[:].rearrange("p (f t) -> p f t", f=4))
                nc.sync.dma_start(bvbc_t[:], bvbc_d[:])
                nc.sync.dma_start(
                    vecs_t[:], vecs_d[:].rearrange("p (v k) -> p v k", v=6))
                nc.sync.dma_start(b1_t[:], b1_d[:])
                nc.sync.dma_start(
                    grows_t[:], grows_d[:].rearrange("o (g e) -> o g e", g=2))
                nc.sync.dma_start(onesr[:], onesr_d[:])
            eps_t = cpool.tile([1, 1], F32, name="eps_t")
            nc.vector.memset(eps_t[:], EPS)
            eps128 = cpool.tile([P, 1], F32, name="eps128")
            nc.vector.memset(eps128[:], EPS)
            bfcrow = cpool.tile([1, E], BF16, name="bfcrow")
            nc.sync.dma_start(bfcrow[:], bfcrow_d[:])
            b2row = cpool.tile([1, E], BF16, name="b2row")
            nc.sync.dma_start(b2row[:], b2row_d[:])

            bfc_t = vecs_t[:, 0, :]
            g1_t = vecs_t[:, 1, :]
            be1_t = vecs_t[:, 2, :]
            b2_t = vecs_t[:, 3, :]
            g2_t = vecs_t[:, 4, :]
            be2_t = vecs_t[:, 5, :]

            old_t = ppool.tile([P, KE, TBLK], BF16, name="old_t")   # LN1 output
            old_tok = ppool.tile([P, 4, E], BF16, name="old_tok")  # same, token-major

            # Prefetch tiles (wfc half + ctxA) — created before the qkv
            # pool so pools close in stack order; DMAs are issued at
            # attention start / mid-attention when the queue is idle.
            es3 = ExitStack()
            pfpool = es3.enter_context(tc.tile_pool(name="pf", bufs=1))
            wfc_a = pfpool.tile([P, 10, E], BF16, name="wfc_a")
            ctxA = pfpool.tile([P, KE, HB], BF16, name="ctxA")

            # q/k/v stay in SBUF between QKV projection and attention.
            qkv_pool = es.enter_context(tc.tile_pool(name="qkv", bufs=1))
            q_sb = qkv_pool.tile([P, 2, 8, 512], BF16, name="q_sb")
            k_sb = qkv_pool.tile([P, 2, 8, 512], BF16, name="k_sb")
            v_sb = qkv_pool.tile([P, 32, CPC], BF16, name="v_sb")

            # ================= Phase Q: q/k/v projections =================
            with (
                tc.tile_pool(name="qw", bufs=1) as qw,
                tc.tile_pool(name="qio", bufs=2) as qio,
                tc.tile_pool(name="qps", bufs=1, space="PSUM") as qps,
            ):
                wq_t = qw.tile([P, KE, CPC], BF16, name="wq_t")
                wq_r = wq_d[:].rearrange("p (k m) -> p k m", k=KE)
                nc.sync.dma_start(wq_t[:, 0:4, :], wq_r[:, 0:4, :])
                nc.sync.dma_start(wq_t[:, 4:, :], wq_r[:, 4:, :])
                wk_t = qw.tile([P, KE, CPC], BF16, name="wk_t")
                wv_t = qw.tile([P, KE, CPC], BF16, name="wv_t")

                for tb in range(8):
                    e_t = qio.tile([P, KE, 512], BF16, tag="emb", bufs=2, name="e_t")
                    e_src = embT_d.ap()[tb].rearrange("p (k t) -> p k t", k=KE)
                    if tb == 0:
                        for q4 in range(4):
                            nc.sync.dma_start(e_t[:, 4 * q4:4 * q4 + 4, :],
                                              e_src[:, 4 * q4:4 * q4 + 4, :])
                    else:
                        nc.sync.dma_start(e_t[:], e_src)
                    if tb == 0:
                        nc.sync.dma_start(bqk_t[:], bqk_d[:])
                        nc.sync.dma_start(
                            wk_t[:], wk_d[:].rearrange("p (k m) -> p k m", k=KE))
                        nc.sync.dma_start(
                            wv_t[:], wv_d[:].rearrange("p (k m) -> p k m", k=KE))
                        load_consts()
                    for wi, (wt, dst) in enumerate(((wq_t, q_sb), (wk_t, k_sb))):
                        for hc in range(2):
                            pqk = qps.tile([P, 512], F32, tag="pqk", bufs=3, name="pqk")
                            for k in range(KE):
                                nc.tensor.matmul(
                                    pqk[:], wt[:, k, hc * P:(hc + 1) * P],
                                    e_t[:, k, :],
                                    start=(k == 0), stop=(k == KE - 1),
                                )
                            nc.scalar.activation(
                                dst[:, hc, tb, :], pqk[:], Identity,
                                bias=bqk_t[:, 2 * wi + hc:2 * wi + hc + 1])
                    for tt in range(4):
                        pv = qps.tile([P, CPC], F32, tag="pv", bufs=3, name="pv")
                        for k in range(KE):
                            nc.tensor.matmul(
                                pv[:], e_t[:, k, tt * P:(tt + 1) * P], wv_t[:, k, :],
                                start=(k == 0), stop=(k == KE - 1),
                            )
                        nc.vector.tensor_add(v_sb[:, 4 * tb + tt, :], pv[:], bvbc_t[:])

            # wfc first half streams in during attention (idle DMA queue)
            nc.sync.dma_start(
                wfc_a[:], wfc_d[0:10].rearrange("k p e -> p k e"))

            # ================= Phase A: causal attention =================
            # scoresT/ctxT per (batch, head), all transposed; softmax denom
            # via ones-matmul; mask added on the PE via identity-matmul
            # accumulation. Diagonal blocks only compute the non-masked
            # column range [j*128, 512); the mask matmul overwrites the
            # fully-masked range (has_written=0 there) with -30000 so the
            # wide Exp still produces exact zeros. ctx is shipped through
            # two half-batch AllToAlls; the batch-0 one fires while
            # batch-1 attention still runs.
            with (
                tc.tile_pool(name="asc", bufs=1) as asc,
                tc.tile_pool(name="aps", bufs=1, space="PSUM") as aps,
            ):
                pending = None

                def finalize(st):
                    b, hc, qt, ctx_ps, l_ps = st
                    l_sb = asc.tile([1, 512], F32, tag="lsb", bufs=3, name="l_sb")
                    nc.vector.tensor_copy(l_sb[:], l_ps[:])
                    r_sb = asc.tile([1, 512], F32, tag="rsb", bufs=3, name="r_sb")
                    nc.vector.reciprocal_approx_fast(r_sb[:], l_sb[:])
                    r_bf = asc.tile([1, 512], BF16, tag="rbf", bufs=3, name="r_bf")
                    nc.vector.tensor_copy(r_bf[:], r_sb[:])
                    rbc_ps = aps.tile([P, 512], F32, tag="sc", bufs=2, name="rbc_ps")
                    nc.tensor.matmul(rbc_ps[:], ones[0:1, :], r_bf[:],
                                     start=True, stop=True)
                    ctx_sb = asc.tile([P, 512], F32, tag="ctxsb", bufs=3,
                                      name="ctx_sb")
                    nc.vector.tensor_copy(ctx_sb[:], ctx_ps[:])
                    ctx_f = asc.tile([P, 512], BF16, tag="ctxf", bufs=4, name="ctx_f")
                    nc.vector.tensor_mul(ctx_f[:], ctx_sb[:], rbc_ps[:])
                    if b == 0:
                        nc.sync.dma_start(
                            a2a0in_d.ap()[2 * qt, hc * P:(hc + 1) * P, :],
                            ctx_f[:, 0:HB])
                        nc.sync.dma_start(
                            a2a0in_d.ap()[2 * qt + 1, hc * P:(hc + 1) * P, :],
                            ctx_f[:, HB:])
                    else:
                        nc.sync.dma_start(a2a1in_d.ap()[2 * qt, hc, :, :],
                                          ctx_f[:, 0:HB])
                        nc.sync.dma_start(a2a1in_d.ap()[2 * qt + 1, hc, :, :],
                                          ctx_f[:, HB:])

                for pi in range(4):
                    b = pi // 2
                    hc = pi % 2
                    qv = q_sb[:, hc, :, :]     # [128, 8, 512]
                    kv = k_sb[:, hc, :, :]
                    v_t = v_sb[:, 16 * b:16 * (b + 1), hc * P:(hc + 1) * P]
                    for qt in range(4):
                        nkb = 4 * qt + 4
                        ctx_ps = aps.tile([P, 512], F32, tag="ctx", bufs=2,
                                          name="ctx_ps")
                        l_ps = aps.tile([1, 512], F32, tag="l", bufs=2, name="l_ps")
                        ex_tiles = [None] * nkb
                        w0s = [max(0, (kb - 4 * qt)) * P for kb in range(nkb)]
                        sc_cur = [None]

                        def emit_sc(kb, qt=qt, kv=kv, qv=qv, b=b,
                                    ex_tiles=ex_tiles, sc_cur=sc_cur, w0s=w0s):
                            # kb-blocks are processed in pairs sharing one
                            # 2-bank psum tile and a single wide Exp.
                            half = kb % 2
                            if half == 0:
                                sc_cur[0] = aps.tile([P, 2, 512], F32, tag="sc",
                                                     bufs=2, name="sc_ps")
                            sc_ps = sc_cur[0]
                            j = kb - 4 * qt
                            diag = j >= 0
                            w0 = w0s[kb]
                            k_sl = kv[:, 4 * b + kb // 4,
                                      (kb % 4) * P:(kb % 4 + 1) * P]
                            nc.tensor.matmul(
                                sc_ps[:, half, w0:512], k_sl,
                                qv[:, 4 * b + qt, w0:512],
                                start=True, stop=not diag)
                            if diag:
                                # overwrites [0:j*128] (has_written=0) with
                                # -30000, accumulates on [j*128:(j+1)*128]
                                nc.tensor.matmul(
                                    sc_ps[:, half, 0:(j + 1) * P], eye[:],
                                    mask_t[:, j, 0:(j + 1) * P],
                                    start=False, stop=True)
                            if half == 1:
                                ex = asc.tile([P, 2, 512], BF16, tag="ex", bufs=3,
                                              name="ex")
                                nc.scalar.activation(ex[:], sc_ps[:], Exp)
                                ex_tiles[kb - 1] = ex[:, 0, :]
                                ex_tiles[kb] = ex[:, 1, :]

                        for w in range(min(4, nkb)):
                            emit_sc(w)
                        for kb in range(nkb):
                            if kb + 4 < nkb:
                                emit_sc(kb + 4)
                            ex = ex_tiles[kb]
                            w0 = w0s[kb]
                            nc.tensor.matmul(ctx_ps[:, w0:512], v_t[:, kb, :],
                                             ex[:, w0:512],
                                             start=(kb == 0), stop=(kb == nkb - 1))
                            nc.tensor.matmul(l_ps[:, w0:512], ones[:, 0:1],
                                             ex[:, w0:512],
                                             start=(kb == 0), stop=(kb == nkb - 1))
                            ex_tiles[kb] = None
                        if pending is not None:
                            pb, phc, pqt = pending[0], pending[1], pending[2]
                            finalize(pending)
                            if (pb, phc, pqt) == (0, 1, 3):
                                # batch 0 fully written -> exchange it while
                                # batch-1 attention continues.
                                nc.gpsimd.collective_compute(
                                    "AllToAll", mybir.AluOpType.bypass,
                                    replica_groups=[list(range(N_CORES))],
                                    ins=[a2a0in_d.ap()], outs=[a2a0out_d.ap()],
                                )
                            elif (pb, phc, pqt) == (1, 0, 3):
                                # a2a0 landed long ago; pull its result into
                                # SBUF while the last head computes (the DMA
                                # queue is past all batch-0 traffic here)
                                nc.sync.dma_start(
                                    ctxA[:],
                                    a2a0out_d.ap()
                                    .rearrange("r (c p) t -> p (r c) t", p=P))
                        pending = (b, hc, qt, ctx_ps, l_ps)
                finalize(pending)

            es.close()   # release q/k/v SBUF

            # ====== AllToAll: all batch-1 ctx (hides under fc half 0) ======
            nc.gpsimd.collective_compute(
                "AllToAll", mybir.AluOpType.bypass,
                replica_groups=[list(range(N_CORES))],
                ins=[a2a1in_d.ap()], outs=[a2a1out_d.ap()],
            )

            # ====== Phase F: transposed fc + residual + LN1 (token-major) ==
            # ctx chunks are stationary, wfc rows stream as the moving
            # operand, so the output lands token-major: LN1 stats come free
            # from accum_out sums, the bias is a rank-1 matmul into PSUM,
            # and the apply is a single per-partition-scalar op per tile
            # (gains 1, betas 0). The e-major old for W1 is built with PE
            # transposes evacuated by the scalar engine.
            with (
                tc.tile_pool(name="fio", bufs=1) as fio,
                tc.tile_pool(name="fps", bufs=1, space="PSUM") as fps,
            ):
                wfc_b = fio.tile([P, 6, E], BF16, name="wfc_b")
                nc.sync.dma_start(
                    wfc_b[:], wfc_d[10:16].rearrange("k p e -> p k e"))
                embres_T = fio.tile([P, 4, E], BF16, name="embres_T")
                nc.sync.dma_start(
                    embres_T[:], embres_d[:].rearrange("p (b e) -> p b e", b=4))
                if not ln_trivial:
                    g1bc = fio.tile([P, E], BF16, name="g1bc")
                    nc.sync.dma_start(g1bc[:], g1bc_d[:])
                    be1bc = fio.tile([P, E], BF16, name="be1bc")
                    nc.sync.dma_start(be1bc[:], be1bc_d[:])
                x_T = fio.tile([P, 4, E], BF16, name="x_T")
                mu1_parts = fio.tile([P, 4, 4], F32, name="mu1_parts")
                sq1_parts = fio.tile([P, 4, 4], F32, name="sq1_parts")
                ctxB = fio.tile([P, KE, HB], BF16, name="ctxB")

                def fc_half(h, ctxH):
                    fps_grp = {}
                    for k in range(KE):
                        wfc_k = wfc_a[:, k, :] if k < 10 else wfc_b[:, k - 10, :]
                        for t2 in range(2):
                            for en in range(4):
                                if k == 0:
                                    fps_grp[(t2, en)] = fps.tile(
                                        [P, 512], F32, tag=f"fps{t2}{en}",
                                        bufs=1, name=f"fps{t2}{en}")
                                nc.tensor.matmul(
                                    fps_grp[(t2, en)][:],
                                    ctxH[:, k, t2 * P:(t2 + 1) * P],
                                    wfc_k[:, en * 512:(en + 1) * 512],
                                    start=(k == 0), stop=False)
                    for t2 in range(2):
                        tb = 2 * h + t2
                        for en in range(4):
                            e0 = en * 512
                            nc.tensor.matmul(
                                fps_grp[(t2, en)][:], ones[0:1, 0:P],
                                bfcrow[0:1, e0:e0 + 512],
                                start=False, stop=True)
                            xsl = x_T[:, tb, e0:e0 + 512]
                            nc.vector.scalar_tensor_tensor(
                                xsl, fps_grp[(t2, en)][:], 1.0,
                                embres_T[:, tb, e0:e0 + 512], MULT, ADD,
                                accum_out=mu1_parts[:, tb, en:en + 1])
                            sqs = fio.tile([P, 512], F32R, tag="sqs1",
                                           bufs=2, name="sqs1")
                            nc.scalar.activation(
                                sqs[:], xsl, Square,
                                accum_out=sq1_parts[:, tb, en:en + 1])

                fc_half(0, ctxA)
                ctxB4 = ctxB[:].rearrange("p (r c) t -> p r c t", c=2)
                nc.sync.dma_start(
                    ctxB4[:, :, :, :],
                    a2a1out_d.ap().rearrange("r c p t -> p r c t"))

                # fold LN1 stats into per-token mean / rstd ([128, 4]);
                # the apply is a single ACT op per tile:
                # out = Identity(x * rstd + (-mean*rstd))
                mean = fio.tile([P, 4], F32, name="mean1")
                var = fio.tile([P, 4], F32, name="var1")
                std4 = fio.tile([P, 4], F32, name="std41")
                rstd4 = fio.tile([P, 4], F32, name="rstd41")
                nmr = fio.tile([P, 4], F32, name="nmr1")
                ta = fio.tile([P, 4], F32, name="ta1")

                def ln1_stats(lo, hi):
                    s = slice(lo, hi)
                    nc.vector.tensor_add(ta[:, s], mu1_parts[:, s, 0],
                                         mu1_parts[:, s, 1])
                    nc.vector.tensor_add(mean[:, s], mu1_parts[:, s, 2],
                                         mu1_parts[:, s, 3])
                    nc.vector.tensor_add(ta[:, s], ta[:, s], mean[:, s])
                    nc.vector.tensor_scalar_mul(mean[:, s], ta[:, s], 1.0 / E)
                    nc.vector.tensor_add(ta[:, s], sq1_parts[:, s, 0],
                                         sq1_parts[:, s, 1])
                    nc.vector.tensor_add(var[:, s], sq1_parts[:, s, 2],
                                         sq1_parts[:, s, 3])
                    nc.vector.tensor_add(ta[:, s], ta[:, s], var[:, s])
                    nc.vector.tensor_scalar_mul(var[:, s], ta[:, s], 1.0 / E)
                    nc.vector.tensor_mul(ta[:, s], mean[:, s], mean[:, s])
                    nc.vector.tensor_sub(var[:, s], var[:, s], ta[:, s])
                    nc.scalar.activation(std4[:, s], var[:, s], Sqrt,
                                         bias=eps128[:])
                    nc.vector.reciprocal_approx_fast(rstd4[:, s], std4[:, s])
                    nc.vector.tensor_mul(nmr[:, s], mean[:, s], rstd4[:, s])
                    nc.vector.tensor_scalar_mul(nmr[:, s], nmr[:, s], -1.0)

                def ln1_apply(tb):
                    for eq in range(4):
                        e0 = eq * 512
                        if ln_trivial:
                            nc.scalar.activation(
                                old_tok[:, tb, e0:e0 + 512],
                                x_T[:, tb, e0:e0 + 512], Identity,
                                bias=nmr[:, tb:tb + 1],
                                scale=rstd4[:, tb:tb + 1])
                        else:
                            t1 = fio.tile([P, 512], BF16, tag="t1f", bufs=3,
                                          name="t1f")
                            nc.vector.scalar_tensor_tensor(
                                t1[:], x_T[:, tb, e0:e0 + 512],
                                mean[:, tb:tb + 1], g1bc[:, e0:e0 + 512],
                                SUB, MULT)
                            nc.vector.scalar_tensor_tensor(
                                old_tok[:, tb, e0:e0 + 512], t1[:],
                                rstd4[:, tb:tb + 1], be1bc[:, e0:e0 + 512],
                                MULT, ADD)

                # token half 0's stats are complete: normalize tb 0/1 on the
                # scalar engine while half 1's matmuls run
                ln1_stats(0, 2)
                ln1_apply(0)
                ln1_apply(1)

                fc_half(1, ctxB)

                ln1_stats(2, 4)
                ln1_apply(2)
                ln1_apply(3)

                # PE-transpose old_tok into e-major old_t, chunk-major so W1
                # can start consuming low-k chunks immediately (DVE evacuates)
                for eq in range(4):
                    for j in range(4):
                        k = eq * 4 + j
                        for tb in range(4):
                            tp = fps.tile([P, P], BF16, tag=f"fps{tb % 2}{j}",
                                          bufs=1, name="tp")
                            nc.tensor.transpose(
                                tp[:],
                                old_tok[:, tb, k * P:(k + 1) * P], eye[:])
                            nc.vector.tensor_copy(
                                old_t[:, k, tb * P:(tb + 1) * P], tp[:])
            es3.close()   # release the wfc/ctxA/embres prefetch SBUF

            # ================= Phase N: FFN + LN2 =================
            # W1: all 64 gelu(h) chunks stay in SBUF (ff-major). The LN1
            # output is also transposed on the PE into token-major old_T
            # (with b2 folded in) for the residual. W2 runs transposed:
            # h chunks are the stationary operand (reused across two
            # 512-wide output blocks), w2 streams as the moving operand,
            # and the output lands token-major so LN2 needs no matmuls at
            # all: stats come from accum_out free-dim sums, the apply is
            # per-partition-scalar work split across DVE and GpSimd.
            with tc.tile_pool(name="nw", bufs=1) as nw:
                h_sb = nw.tile([P, 64, TBLK], BF16, name="h_sb")
                if not ln_trivial:
                    g2bc = nw.tile([P, E], BF16, name="g2bc")
                    nc.sync.dma_start(g2bc[:], g2bc_d[:])
                    be2bc = nw.tile([P, E], BF16, name="be2bc")
                    nc.sync.dma_start(be2bc[:], be2bc_d[:])
                with tc.tile_pool(name="nps1", bufs=1, space="PSUM") as nps1:
                    # first 8 rows run k-interleaved across 8 PSUM banks so
                    # they consume old_t chunks as the LN1 transposes land
                    grp = {}
                    w1g = {}
                    for hb in range(8):
                        w1g[hb] = nw.tile([P, KE, P], BF16, tag="w1", bufs=10,
                                          name="w1_t")
                        nc.sync.dma_start(
                            w1g[hb][:],
                            w1_d.ap()[hb].rearrange("p (k m) -> p k m", k=KE))
                        grp[hb] = nps1.tile([P, 512], F32, tag=f"hps{hb}",
                                            bufs=1, name=f"hps{hb}")
                    for k in range(KE):
                        for hb in range(8):
                            nc.tensor.matmul(grp[hb][:], w1g[hb][:, k, :],
                                             old_t[:, k, :],
                                             start=(k == 0), stop=(k == KE - 1))
                    for hb in range(8):
                        nc.scalar.activation(h_sb[:, hb, :], grp[hb][:], Gelu,
                                             bias=b1_t[:, hb:hb + 1])
                    for hb in range(8, 64):
                        w1_t = nw.tile([P, KE, P], BF16, tag="w1", bufs=10,
                                       name="w1_t")
                        nc.sync.dma_start(
                            w1_t[:],
                            w1_d.ap()[hb].rearrange("p (k m) -> p k m", k=KE))
                        hps = nps1.tile([P, 512], F32, tag=f"hps{hb % 8}",
                                        bufs=1, name="hps")
                        for k in range(KE):
                            nc.tensor.matmul(hps[:], w1_t[:, k, :], old_t[:, k, :],
                                             start=(k == 0), stop=(k == KE - 1))
                        nc.scalar.activation(h_sb[:, hb, :], hps[:], Gelu,
                                             bias=b1_t[:, hb:hb + 1])

                x2_t = nw.tile([P, 4, E], BF16, name="x2_t")
                mu_parts = nw.tile([P, 4, 4], F32, name="mu_parts")
                sq_parts = nw.tile([P, 4, 4], F32, name="sq_parts")
                mean2 = nw.tile([P, 4], F32, name="mean2")
                var2 = nw.tile([P, 4], F32, name="var2")
                std2 = nw.tile([P, 4], F32, name="std2")
                rstd2 = nw.tile([P, 4], F32, name="rstd2")
                nmr2 = nw.tile([P, 4], F32, name="nmr2")
                tb2s = nw.tile([P, 4], F32, name="tb2s")
                with tc.tile_pool(name="nps2", bufs=1, space="PSUM") as nps:
                    yps = {}
                    for eh in range(2):
                        for hl in range(64):
                            w2_t = nw.tile([P, 8 * P], BF16, tag="w2", bufs=4,
                                           name="w2_t")
                            nc.sync.dma_start(w2_t[:], w2_d.ap()[eh, hl])
                            for tb in range(4):
                                for en in range(2):
                                    if hl == 0:
                                        yps[(tb, en)] = nps.tile(
                                            [P, 512], F32, tag=f"yps{tb}{en}",
                                            bufs=1, name=f"yps{tb}{en}")
                                    nc.tensor.matmul(
                                        yps[(tb, en)][:],
                                        h_sb[:, hl, tb * P:(tb + 1) * P],
                                        w2_t[:, en * 512:(en + 1) * 512],
                                        start=(hl == 0), stop=False)
                        for tb in range(4):
                            for en in range(2):
                                e0 = eh * 1024 + en * 512
                                nc.tensor.matmul(
                                    yps[(tb, en)][:], ones[0:1, 0:P],
                                    b2row[0:1, e0:e0 + 512],
                                    start=False, stop=True)
                                xsl = x2_t[:, tb, e0:e0 + 512]
                                pi = 2 * eh + en
                                nc.vector.scalar_tensor_tensor(
                                    xsl, yps[(tb, en)][:], 1.0,
                                    old_tok[:, tb, e0:e0 + 512], MULT, ADD,
                                    accum_out=mu_parts[:, tb, pi:pi + 1])
                                sqs = nw.tile([P, 512], F32R, tag="sqs",
                                              bufs=2, name="sqs")
                                # split the squares between DVE and ACT so
                                # neither engine owns the whole tail
                                if (eh == 1) and (en == 0):
                                    nc.vector.scalar_tensor_tensor(
                                        sqs[:], xsl, 1.0, xsl, MULT, MULT,
                                        accum_out=sq_parts[:, tb, pi:pi + 1])
                                else:
                                    nc.scalar.activation(
                                        sqs[:], xsl, Square,
                                        accum_out=sq_parts[:, tb, pi:pi + 1])
                            if eh == 1:
                                # this token block's stats are complete:
                                # fold and normalize it right away
                                s = slice(tb, tb + 1)
                                nc.vector.tensor_add(
                                    tb2s[:, s], mu_parts[:, s, 0],
                                    mu_parts[:, s, 1])
                                nc.vector.tensor_add(
                                    mean2[:, s], mu_parts[:, s, 2],
                                    mu_parts[:, s, 3])
                                nc.vector.tensor_add(tb2s[:, s], tb2s[:, s],
                                                     mean2[:, s])
                                nc.vector.tensor_scalar_mul(
                                    mean2[:, s], tb2s[:, s], 1.0 / E)
                                nc.vector.tensor_add(
                                    tb2s[:, s], sq_parts[:, s, 0],
                                    sq_parts[:, s, 1])
                                nc.vector.tensor_add(
                                    var2[:, s], sq_parts[:, s, 2],
                                    sq_parts[:, s, 3])
                                nc.vector.tensor_add(tb2s[:, s], tb2s[:, s],
                                                     var2[:, s])
                                nc.vector.tensor_scalar_mul(
                                    var2[:, s], tb2s[:, s], 1.0 / E)
                                nc.vector.tensor_mul(tb2s[:, s], mean2[:, s],
                                                     mean2[:, s])
                                nc.vector.tensor_sub(var2[:, s], var2[:, s],
                                                     tb2s[:, s])
                                nc.scalar.activation(std2[:, s], var2[:, s],
                                                     Sqrt, bias=eps128[:])
                                nc.vector.reciprocal_approx_fast(
                                    rstd2[:, s], std2[:, s])
                                nc.vector.tensor_mul(nmr2[:, s], mean2[:, s],
                                                     rstd2[:, s])
                                nc.vector.tensor_scalar_mul(
                                    nmr2[:, s], nmr2[:, s], -1.0)
                                for eq in range(4):
                                    e0 = eq * 512
                                    oc = nw.tile([P, 512], F32, tag="ocd",
                                                 bufs=4, name="oc")
                                    if ln_trivial:
                                        nc.scalar.activation(
                                            oc[:], x2_t[:, tb, e0:e0 + 512],
                                            Identity,
                                            bias=nmr2[:, tb:tb + 1],
                                            scale=rstd2[:, tb:tb + 1])
                                    else:
                                        t2 = nw.tile([P, 512], BF16,
                                                     tag="t2d", bufs=2,
                                                     name="t2")
                                        nc.vector.scalar_tensor_tensor(
                                            t2[:], x2_t[:, tb, e0:e0 + 512],
                                            mean2[:, tb:tb + 1],
                                            g2bc[:, e0:e0 + 512], SUB, MULT)
                                        nc.vector.scalar_tensor_tensor(
                                            oc[:], t2[:],
                                            rstd2[:, tb:tb + 1],
                                            be2bc[:, e0:e0 + 512], MULT, ADD)
                                    nc.sync.dma_start(
                                        out_d.ap()[tb][:, e0:e0 + 512], oc[:])

    nc.compile()
    return nc


@functools.lru_cache(maxsize=2)
def _get_program(ln_trivial=True):
    return _build_program(ln_trivial)


BF = ml_dtypes.bfloat16


def _pack_w(w):
    """[E_rows, M] f32 -> [128, (E_rows/128)*M] bf16 with [p, k, m] layout."""
    e, m = w.shape
    return np.ascontiguousarray(
        w.reshape(e // P, P, m).transpose(1, 0, 2).reshape(P, -1).astype(BF))


def _pack_vec(v):
    """[n*128] -> [128, n] per-partition chunks."""
    return np.ascontiguousarray(v.reshape(-1, P).T)


def _prepare_in_maps(inputs, ln_trivial=True):
    f32 = np.float32
    emb = np.asarray(inputs["embeddings"], f32).reshape(T, E)
    embT = np.ascontiguousarray(emb.T)
    embT_bf = np.ascontiguousarray(
        embT.reshape(KE, P, 8, 512).transpose(2, 1, 0, 3)
        .reshape(8, P, KE * 512).astype(BF))
    scale = 1.0 / math.sqrt(HD)

    Wq = np.asarray(inputs["Wq"], f32)
    Wk = np.asarray(inputs["Wk"], f32)
    Wv = np.asarray(inputs["Wv"], f32)
    bq = np.asarray(inputs["bq"], f32)
    bk = np.asarray(inputs["bk"], f32)
    bv = np.asarray(inputs["bv"], f32)
    Wfc = np.asarray(inputs["Wfc"], f32)
    W1 = np.asarray(inputs["W1"], f32)
    W2 = np.asarray(inputs["W2"], f32)

    vecs = np.concatenate([
        _pack_vec(np.asarray(inputs[n], f32))
        for n in ("bfc", "g1", "be1", "b2", "g2", "be2")
    ], axis=1)  # [128, 6*KE]

    wfcp = np.ascontiguousarray(Wfc.reshape(KE, P, E).astype(BF))
    w1p = np.ascontiguousarray(
        W1.reshape(KE, P, 64, P).transpose(2, 1, 0, 3)
        .reshape(64, P, KE * P).astype(BF))
    w2p = np.ascontiguousarray(
        W2.reshape(64, P, 2, 8 * P).transpose(2, 0, 1, 3).astype(BF))
    b1p = np.ascontiguousarray(np.asarray(inputs["b1"], f32).reshape(64, P).T)

    j = np.arange(P)[:, None, None]
    pp = np.arange(4)[None, :, None]
    cc = np.arange(512)[None, None, :]
    maskT = np.where(P * pp + j <= cc, 0.0, -30000.0).astype(BF).reshape(P, 4 * 512)
    onesblk = np.ones((P, P), BF)
    eyeblk = np.eye(P, dtype=BF)
    grows = np.concatenate([np.asarray(inputs["g1"], f32),
                            np.asarray(inputs["g2"], f32)]) \
        .reshape(1, 2 * KE * P).astype(BF)

    in_maps = []
    for c in range(N_CORES):
        sl = slice(CPC * c, CPC * (c + 1))
        bqs = (bq[sl] * scale).reshape(2, P).T
        bks = bk[sl].reshape(2, P).T
        in_maps.append({
            "embT": embT_bf,
            "embres": np.ascontiguousarray(
                np.concatenate(
                    [emb[256 * c:256 * (c + 1)],
                     emb[S + 256 * c:S + 256 * (c + 1)]], axis=0)
                .reshape(4, P, E).transpose(1, 0, 2)
                .reshape(P, 4 * E).astype(BF)),
            "wq": _pack_w(Wq[:, sl] * scale),
            "wk": _pack_w(Wk[:, sl]),
            "wv": _pack_w(Wv[:, sl]),
            "bqk": np.ascontiguousarray(np.concatenate([bqs, bks], axis=1)),
            "bvbc": np.ascontiguousarray(np.broadcast_to(bv[sl], (P, CPC)).copy()),
            "wfc": wfcp,
            "vecs": vecs,
            "w1": w1p,
            "b1": b1p,
            "w2": w2p,
            "maskT": maskT,
            "onesblk": onesblk,
            "onesr": np.ones((P, P), np.float32),
            "bfcrow": np.ascontiguousarray(
                np.asarray(inputs["bfc"], f32).reshape(1, E).astype(BF)),
            "b2row": np.ascontiguousarray(
                np.asarray(inputs["b2"], f32).reshape(1, E).astype(BF)),

            "eyeblk": eyeblk,
            "grows": grows,
        })
        if not ln_trivial:
            for nm, vec in (("g1bc", "g1"), ("be1bc", "be1"),
                            ("g2bc", "g2"), ("be2bc", "be2")):
                in_maps[-1][nm] = np.ascontiguousarray(
                    np.broadcast_to(np.asarray(inputs[vec], f32), (P, E))
                    .astype(BF))
    return in_maps


def _ln_trivial(inputs):
    return bool(
        np.allclose(np.asarray(inputs["g1"]), 1.0)
        and np.allclose(np.asarray(inputs["g2"]), 1.0)
        and np.allclose(np.asarray(inputs["be1"]), 0.0)
        and np.allclose(np.asarray(inputs["be2"]), 0.0))


def kernel(**inputs) -> np.ndarray:
    ln_trivial = _ln_trivial(inputs)
    nc = _get_program(ln_trivial)
    in_maps = _prepare_in_maps(inputs, ln_trivial)
    res = None
    last_err = None
    for attempt in range(3):
        try:
            res = run_bass_kernel_spmd(nc, in_maps, core_ids=list(range(N_CORES)))
            break
        except Exception as e:  # transient device/runtime hiccup: retry
            last_err = e
            import time as _time
            _time.sleep(3.0)
    if res is None:
        raise last_err
    out = np.empty((T, E), dtype=np.float32)
    for c in range(N_CORES):
        toks = res.results[c]["outp"].reshape(TBLK, E)  # token-major
        out[256 * c:256 * (c + 1)] = toks[0:256]
        out[S + 256 * c:S + 256 * (c + 1)] = toks[256:512]
    return np.ascontiguousarray(out.reshape(B, S, E))
